# revision 11
# baseline (speedup 1.0000x reference)
"""SIR ODE batch integrator on 8 Trainium2 NeuronCores (Bass/Tile).

Problem: for each of B=65536 samples with params (beta, gamma, S0, I0),
integrate the SIR system dS=-bSI, dI=bSI-gI, dR=gI over 199 fixed
intervals (t = linspace(0,100,200), fp32) and return the trajectory
[B, 200, 3].

Strategy:
  - Pure data parallel: 8192 samples per core, laid out as [128 part, 64 free].
  - 2-state formulation: integrate (S, C) with C = S + I (R = 1 - C,
    I = C - S recovered on host).  The stage derivative K = [-b*t | -g*I]
    (t = S*I) is produced by one custom DVE op (X = [t | I] from the state
    and its column-block-swapped view) plus one multiply against the
    per-sample constants [-beta | -gamma].
  - Integration schedule (validated numerically against the fp32 reference
    in sched_sim.py): interval 0 = midpoint RK2 with 2 substeps (seeds the
    multistep history), intervals 1-8 = variable-step Adams-Bashforth-2
    with 2 substeps (fast early transients), later intervals = single-step
    AB2.  One derivative eval per AB2 step; 4 logical DVE ops per step.
    Measured vs the fp32 reference: rel fro-norm ~2.4e-3, absmax ~1.7e-2
    (gate is rel < 2e-2).
  - Dual-chain interleave: the DVE pays a ~100ns read-after-write stall on
    back-to-back dependent instructions, so each core's samples are split
    into two column-half chains (A = per-partition samples 0:32, B = 32:64)
    whose op streams interleave.  Consecutive instructions are independent
    (same-chain dependency distance 2), hiding the stall: measured ~16%
    faster than the single full-width chain.  Per-chain state is
    [S_x | C_x] (64 cols); pin/stage layouts are [A-block | B-block].
  - Output: interval results are written into a [128, G*128] staging tile
    (G=4 intervals per buffer); one DMA per full buffer into
    out[199,128,128].  This cuts SP-sequencer DMA-issue time ~4x vs
    per-interval DMAs.  Host unpacks, computes I and R, and transposes
    into [B,200,3].
"""

import numpy as np

try:
    import concourse.bass as bass
except ImportError:  # pragma: no cover - container default location
    import sys

    sys.path.insert(0, "/opt/trn_rl_repo")
    import concourse.bass as bass

import concourse.bacc as bacc
import concourse.mybir as mybir
from concourse.tile import TileContext
from concourse.bass_utils import run_bass_kernel_spmd

F32 = mybir.dt.float32
AL = mybir.AluOpType


def _register_ti_op():
    """Register a custom DVE op computing X = [t | I] from Y = [S | C] in ONE
    wide instruction: in0 = Y, in1 = column-block-swapped Y (= [C | S]).
    With r = Src1 - Src0:
      k <  64 (Src0=S, Src1=C): out = r*Src0 = (C-S)*S = S*I   (t half)
      k >= 64 (Src0=C, Src1=S): out = 0-r    = C-S = I         (I half)
    Bit-identical to the separate subtract+mult pair it replaces."""
    import numpy as _np
    from concourse import dve_ops as _dve_ops
    from concourse.dve_spec import Spec, Src0, Src1, C0, Zero, Idx, select, lower
    from concourse.dve_uop import DveOpSpec

    name = "SIR_TI_FUSED"
    for op in _dve_ops.OPS:
        if op.name == name:
            return op
    r = Src1 - Src0

    def _ref(in0, in1, s0):
        idx = _np.arange(in0.shape[-1], dtype=_np.float32)
        rr = in1 - in0
        return _np.where(idx < s0, rr * in0, -rr)

    spec = Spec(body=select(Idx < C0, r * Src0, Zero - r), reference=_ref)
    row = _dve_ops._CUSTOM_DVE_ROW_BASE + len(_dve_ops.OPS)
    assert row < 0x20
    shas = {
        ver: DveOpSpec(
            name=name, opcode=row, uops=lower(spec, ver=ver), rd1_en=True
        ).sha(ver)
        for ver in ("v3", "v4")
    }
    op = _dve_ops.DveOp(name, spec, subdim=False, uops_sha=shas)
    _dve_ops.OPS.append(op)
    _dve_ops.CUSTOM_DVE_SPECS[name] = spec
    _dve_ops._SUB_OPCODE_FOR_NAME[name] = row
    return op


_TI_OP = _register_ti_op()

N_CORES = 8
B = 65536
PER = B // N_CORES  # 8192 samples per core
P = 128
F = PER // P  # 64
NUM_T = 200
NI = NUM_T - 1  # 199 intervals
G = 8  # intervals per output staging buffer / DMA

# Bit-exact fp32 dt values of jnp.linspace(0, 100, 200, float32) diffs.
_DT_BITS = [
    0x3F00A4AA, 0x3F00A4AA, 0x3F00A4AA, 0x3F00A4AA, 0x3F00A4A8, 0x3F00A4AC, 0x3F00A4AC, 0x3F00A4A8, 0x3F00A4A8, 0x3F00A4A8,
    0x3F00A4B0, 0x3F00A4A8, 0x3F00A4A8, 0x3F00A4B0, 0x3F00A4A8, 0x3F00A4A8, 0x3F00A4B0, 0x3F00A4A0, 0x3F00A4B0, 0x3F00A4A0,
    0x3F00A4B0, 0x3F00A4B0, 0x3F00A4A0, 0x3F00A4B0, 0x3F00A4B0, 0x3F00A4A0, 0x3F00A4B0, 0x3F00A4B0, 0x3F00A4A0, 0x3F00A4B0,
    0x3F00A4A0, 0x3F00A4B0, 0x3F00A4A0, 0x3F00A4C0, 0x3F00A4A0, 0x3F00A4A0, 0x3F00A4C0, 0x3F00A4A0, 0x3F00A4A0, 0x3F00A4A0,
    0x3F00A4C0, 0x3F00A4A0, 0x3F00A4A0, 0x3F00A4C0, 0x3F00A4A0, 0x3F00A4A0, 0x3F00A4C0, 0x3F00A4A0, 0x3F00A4A0, 0x3F00A4C0,
    0x3F00A4A0, 0x3F00A4A0, 0x3F00A4C0, 0x3F00A4A0, 0x3F00A4A0, 0x3F00A4C0, 0x3F00A4A0, 0x3F00A4A0, 0x3F00A4A0, 0x3F00A4C0,
    0x3F00A4A0, 0x3F00A4A0, 0x3F00A4C0, 0x3F00A4A0, 0x3F00A4C0, 0x3F00A480, 0x3F00A4C0, 0x3F00A4C0, 0x3F00A480, 0x3F00A4C0,
    0x3F00A4C0, 0x3F00A480, 0x3F00A4C0, 0x3F00A4C0, 0x3F00A480, 0x3F00A4C0, 0x3F00A4C0, 0x3F00A480, 0x3F00A4C0, 0x3F00A480,
    0x3F00A4C0, 0x3F00A4C0, 0x3F00A480, 0x3F00A4C0, 0x3F00A4C0, 0x3F00A480, 0x3F00A4C0, 0x3F00A4C0, 0x3F00A480, 0x3F00A4C0,
    0x3F00A4C0, 0x3F00A480, 0x3F00A4C0, 0x3F00A4C0, 0x3F00A480, 0x3F00A4C0, 0x3F00A4C0, 0x3F00A480, 0x3F00A4C0, 0x3F00A4C0,
    0x3F00A480, 0x3F00A4C0, 0x3F00A4C0, 0x3F00A480, 0x3F00A4C0, 0x3F00A4C0, 0x3F00A480, 0x3F00A4C0, 0x3F00A4C0, 0x3F00A480,
    0x3F00A4C0, 0x3F00A4C0, 0x3F00A480, 0x3F00A4C0, 0x3F00A480, 0x3F00A4C0, 0x3F00A4C0, 0x3F00A480, 0x3F00A4C0, 0x3F00A4C0,
    0x3F00A480, 0x3F00A4C0, 0x3F00A4C0, 0x3F00A480, 0x3F00A4C0, 0x3F00A4C0, 0x3F00A480, 0x3F00A4C0, 0x3F00A480, 0x3F00A500,
    0x3F00A480, 0x3F00A480, 0x3F00A500, 0x3F00A480, 0x3F00A480, 0x3F00A500, 0x3F00A480, 0x3F00A480, 0x3F00A500, 0x3F00A480,
    0x3F00A480, 0x3F00A500, 0x3F00A480, 0x3F00A480, 0x3F00A500, 0x3F00A480, 0x3F00A480, 0x3F00A500, 0x3F00A480, 0x3F00A480,
    0x3F00A500, 0x3F00A480, 0x3F00A480, 0x3F00A500, 0x3F00A480, 0x3F00A480, 0x3F00A500, 0x3F00A480, 0x3F00A480, 0x3F00A480,
    0x3F00A500, 0x3F00A480, 0x3F00A480, 0x3F00A500, 0x3F00A480, 0x3F00A480, 0x3F00A500, 0x3F00A480, 0x3F00A480, 0x3F00A500,
    0x3F00A480, 0x3F00A480, 0x3F00A500, 0x3F00A480, 0x3F00A480, 0x3F00A500, 0x3F00A480, 0x3F00A480, 0x3F00A500, 0x3F00A480,
    0x3F00A480, 0x3F00A500, 0x3F00A480, 0x3F00A480, 0x3F00A500, 0x3F00A480, 0x3F00A480, 0x3F00A500, 0x3F00A480, 0x3F00A480,
    0x3F00A500, 0x3F00A480, 0x3F00A480, 0x3F00A500, 0x3F00A480, 0x3F00A480, 0x3F00A500, 0x3F00A480, 0x3F00A480,
]
DTS = np.array(_DT_BITS, dtype=np.uint32).view(np.float32)
assert DTS.shape == (NI,)

# Integration schedule: (method, substeps) per interval.  See module
# docstring; error vs fp32 reference rel ~2.4e-3 / absmax ~1.7e-2
# (sched_sim.py), gate is rel < 2e-2.
AB2S2 = 8
SCHEDULE = [("mid", 2)] + [("ab2", 2)] * AB2S2 + [("ab2", 1)] * (NI - 1 - AB2S2)
assert len(SCHEDULE) == NI

H = F // 2  # 32 cols per chain per state variable


def _rev(ap):
    return ap.rearrange("p (two f) -> p two f", two=2)[:, ::-1, :]


def _eval_K(nc, pool, cstA, cstB, YA, YB, tag):
    """Stage derivative K = [-b*S*I | -g*I] for both chains (4 DVE ops,
    interleaved A/B so consecutive instructions are independent)."""
    v = nc.vector
    X = pool.tile([P, 2 * F], F32, tag="X")
    XA, XB = X[:, 0 : 2 * H], X[:, 2 * H : 4 * H]
    v._custom_dve(_TI_OP, out=XA, in0=YA, in1=_rev(YA), s0=float(H))  # [t | I]
    v._custom_dve(_TI_OP, out=XB, in0=YB, in1=_rev(YB), s0=float(H))
    K = pool.tile([P, 2 * F], F32, tag=tag)
    v.scalar_tensor_tensor(K[:, 0 : 2 * H], cstA, 1.0, XA, AL.mult, AL.mult)
    v.scalar_tensor_tensor(K[:, 2 * H : 4 * H], cstB, 1.0, XB, AL.mult, AL.mult)
    return K


def _sub_mid(nc, pool, cstA, cstB, YA, YB, YoutA, YoutB, h):
    """Midpoint RK2, both chains: 12 DVE ops.  Returns f(Y) (AB2 history)."""
    v = nc.vector
    c2 = float(h / np.float32(2.0))
    K1 = _eval_K(nc, pool, cstA, cstB, YA, YB, "Kab")
    Y2 = pool.tile([P, 2 * F], F32, tag="Y2")
    Y2A, Y2B = Y2[:, 0 : 2 * H], Y2[:, 2 * H : 4 * H]
    v.scalar_tensor_tensor(Y2A, K1[:, 0 : 2 * H], c2, YA, AL.mult, AL.add)
    v.scalar_tensor_tensor(Y2B, K1[:, 2 * H : 4 * H], c2, YB, AL.mult, AL.add)
    K2 = _eval_K(nc, pool, cstA, cstB, Y2A, Y2B, "K2")
    v.scalar_tensor_tensor(YoutA, K2[:, 0 : 2 * H], float(h), YA, AL.mult, AL.add)
    v.scalar_tensor_tensor(YoutB, K2[:, 2 * H : 4 * H], float(h), YB, AL.mult, AL.add)
    return K1


def _sub_ab2(nc, pool, cstA, cstB, YA, YB, YoutA, YoutB, kprev, a, brat):
    """Variable-step Adams-Bashforth 2, both chains: 8 DVE ops.
    y+ = y + a*(k_n + brat*k_{n-1}),  a = h_n(1+r/2), brat = -(r/2)/(1+r/2),
    r = h_n/h_{n-1}.  Returns k_n (next step's history)."""
    v = nc.vector
    K = _eval_K(nc, pool, cstA, cstB, YA, YB, "Kab")
    Bt = pool.tile([P, 2 * F], F32, tag="B")
    v.scalar_tensor_tensor(
        Bt[:, 0 : 2 * H], kprev[:, 0 : 2 * H], brat, K[:, 0 : 2 * H],
        AL.mult, AL.add,
    )
    v.scalar_tensor_tensor(
        Bt[:, 2 * H : 4 * H], kprev[:, 2 * H : 4 * H], brat, K[:, 2 * H : 4 * H],
        AL.mult, AL.add,
    )
    v.scalar_tensor_tensor(YoutA, Bt[:, 0 : 2 * H], a, YA, AL.mult, AL.add)
    v.scalar_tensor_tensor(YoutB, Bt[:, 2 * H : 4 * H], a, YB, AL.mult, AL.add)
    return K


def _emit_body(nc, cpool, spool, wpool, pin, out_fn):
    """Emit one full integration.  out_fn(k0, width_ivals, stage_tile) is
    called to emit the output DMA for intervals [k0, k0+width_ivals)."""

    def body(_=None):
        pint = cpool.tile([P, 4 * F], F32, tag="pin")
        nc.sync.dma_start(out=pint[:], in_=pin[:])
        cstA = pint[:, 0 : 2 * H]  # [-bA | -gA]
        cstB = pint[:, 2 * H : 4 * H]  # [-bB | -gB]
        YA = pint[:, 4 * H : 6 * H]  # [SA | CA]
        YB = pint[:, 6 * H : 8 * H]  # [SB | CB]
        kprev = None
        h_prev = None
        stage = None
        k0 = 0
        for k in range(NI):
            slot = k % G
            if slot == 0:
                stage = spool.tile([P, G * 2 * F], F32, tag="stage")
                k0 = k
            base = slot * 2 * F
            YsA = stage[:, base : base + 2 * H]
            YsB = stage[:, base + 2 * H : base + 4 * H]
            meth, nsub = SCHEDULE[k]
            h = np.float32(DTS[k]) / np.float32(nsub)
            for s in range(nsub):
                if s == nsub - 1:
                    YoA, YoB = YsA, YsB
                else:
                    Ymid = wpool.tile([P, 2 * F], F32, tag="Ymid")
                    YoA, YoB = Ymid[:, 0 : 2 * H], Ymid[:, 2 * H : 4 * H]
                if meth == "mid":
                    kprev = _sub_mid(nc, wpool, cstA, cstB, YA, YB, YoA, YoB, h)
                    h_prev = float(h)
                elif meth == "ab2":
                    hn = float(h)
                    r = hn / h_prev
                    a = float(np.float32(hn * (1 + r / 2)))
                    brat = float(np.float32(-(r / 2) / (1 + r / 2)))
                    kprev = _sub_ab2(
                        nc, wpool, cstA, cstB, YA, YB, YoA, YoB, kprev, a, brat
                    )
                    h_prev = hn
                else:
                    raise ValueError(meth)
                YA, YB = YoA, YoB
            if slot == G - 1 or k == NI - 1:
                out_fn(k0, slot + 1, stage)

    return body


def _declare_and_build(nc, reps, out_dram, live_out=None):
    pin = nc.declare_dram_parameter("pin", [P, 4 * F], F32, isOutput=False)

    with TileContext(nc) as tc:
        with (
            tc.tile_pool(name="const", bufs=1) as cpool,
            tc.tile_pool(name="stage", bufs=3) as spool,
            tc.tile_pool(name="work", bufs=3) as wpool,
        ):

            def out_fn(k0, nival, stage):
                # keep the partition axis outermost on the SBUF side; the
                # DRAM side takes the matching strided view
                src = stage[:, 0 : nival * 2 * F].rearrange(
                    "p (g f) -> p g f", g=nival
                )
                dst = out_dram[k0 : k0 + nival].rearrange("g p f -> p g f")
                nc.sync.dma_start(out=dst, in_=src)

            body = _emit_body(nc, cpool, spool, wpool, pin, out_fn)
            if reps == 1:
                body()
            else:
                # timing mode: repeat the whole kernel body inside one NEFF so
                # per-rep HW time can be separated from dispatch overhead
                with tc.For_i(0, reps, 1):
                    body()
            if live_out is not None:
                # graph liveness for the timing twin: route one scratch tile
                # to the tiny external output after the loop
                fin = cpool.tile([P, 2 * F], F32, tag="fin")
                nc.sync.dma_start(out=fin[:], in_=out_dram[NI - 1])
                nc.sync.dma_start(out=live_out[:], in_=fin[:])
    # run_bass_via_pjrt does not finalize; Bacc needs it (register alloc +
    # sync-wait splitting happen in its compile() pipeline).
    nc.finalize()
    return nc


def build_nc(reps=1):
    # Bacc (not raw Bass): its compile() pipeline runs generate_event_semaphores,
    # which splits multi-wait sync conditions that TRN2 instructions can't carry.
    nc = bacc.Bacc(None)
    out = nc.declare_dram_parameter("out", [NI, P, 2 * F], F32, isOutput=True)
    return _declare_and_build(nc, reps, out)


def build_timing(reps):
    """Timing twin: trajectory goes to internal DRAM scratch (no 104MB host
    zero-out transfer per call), tiny external output keeps the graph live."""
    nc = bacc.Bacc(None)
    outs = nc.declare_dram_parameter("outs", [P, 2 * F], F32, isOutput=True)
    scratch = nc.dram_tensor("traj", [NI, P, 2 * F], F32, kind="Internal")
    return _declare_and_build(nc, reps, scratch, live_out=outs)


_NC_CACHE = {}


def kernel(params: np.ndarray) -> np.ndarray:
    params = np.asarray(params, dtype=np.float32)
    assert params.shape == (B, 4)

    if "nc" not in _NC_CACHE:
        _NC_CACHE["nc"] = build_nc()
    nc = _NC_CACHE["nc"]

    in_maps = []
    for c in range(N_CORES):
        sl = params[c * PER : (c + 1) * PER]
        rb = (-sl[:, 0]).reshape(P, F)  # -beta
        rg = (-sl[:, 1]).reshape(P, F)  # -gamma
        rs = sl[:, 2].reshape(P, F)  # S0
        rc = (sl[:, 2] + sl[:, 3]).reshape(P, F)  # C0 = S0+I0
        # chain-blocked layout: [bA|gA|bB|gB | SA|CA|SB|CB], blocks of H=32
        Hh = F // 2
        pin = np.empty((P, 4 * F), dtype=np.float32)
        pin[:, 0:Hh] = rb[:, :Hh]
        pin[:, Hh : 2 * Hh] = rg[:, :Hh]
        pin[:, 2 * Hh : 3 * Hh] = rb[:, Hh:]
        pin[:, 3 * Hh : 4 * Hh] = rg[:, Hh:]
        pin[:, 4 * Hh : 5 * Hh] = rs[:, :Hh]
        pin[:, 5 * Hh : 6 * Hh] = rc[:, :Hh]
        pin[:, 6 * Hh : 7 * Hh] = rs[:, Hh:]
        pin[:, 7 * Hh : 8 * Hh] = rc[:, Hh:]
        in_maps.append({"pin": pin})

    res = run_bass_kernel_spmd(nc, in_maps, list(range(N_CORES)))

    out_full = np.empty((B, NUM_T, 3), dtype=np.float32)
    one = np.float32(1.0)
    S0 = params[:, 2]
    I0 = params[:, 3]
    out_full[:, 0, 0] = S0
    out_full[:, 0, 1] = I0
    out_full[:, 0, 2] = (one - S0) - I0
    for c in range(N_CORES):
        o = res.results[c]["out"]  # [NI, P, 2F] = [.., SA|CA|SB|CB]
        ob = o.reshape(NI, P, 4, F // 2)
        S = ob[:, :, (0, 2), :].reshape(NI, PER).T  # [PER, NI]
        C = ob[:, :, (1, 3), :].reshape(NI, PER).T
        blk = out_full[c * PER : (c + 1) * PER]
        blk[:, 1:, 0] = S
        blk[:, 1:, 1] = C - S
        blk[:, 1:, 2] = one - C
    return out_full


if __name__ == "__main__":
    rng = np.random.RandomState(0)
    p = rng.uniform(0, 1, (B, 4)).astype(np.float32)
    r = kernel(p)
    print(r.shape, r.dtype, r[0, :3], flush=True)


# revision 12
# speedup vs baseline: 1.0491x; 1.0491x over previous
"""SIR ODE batch integrator on 8 Trainium2 NeuronCores (Bass/Tile).

Problem: for each of B=65536 samples with params (beta, gamma, S0, I0),
integrate the SIR system dS=-bSI, dI=bSI-gI, dR=gI over 199 fixed
intervals (t = linspace(0,100,200), fp32) and return the trajectory
[B, 200, 3].

Strategy:
  - Pure data parallel: 8192 samples per core, laid out as [128 part, 64 free].
  - 2-state formulation: integrate (S, C) with C = S + I (R = 1 - C,
    I = C - S recovered on host).  The stage derivative K = [-b*t | -g*I]
    (t = S*I) is produced by one custom DVE op (X = [t | I] from the state
    and its column-block-swapped view) plus one multiply against the
    per-sample constants [-beta | -gamma].
  - Integration schedule (validated numerically against the fp32 reference
    in sched_sim.py): interval 0 = midpoint RK2 with 2 substeps (seeds the
    multistep history), intervals 1-8 = variable-step Adams-Bashforth-2
    with 2 substeps (fast early transients), later intervals = single-step
    AB2.  One derivative eval per AB2 step; 4 logical DVE ops per step.
    Measured vs the fp32 reference: rel fro-norm ~2.4e-3, absmax ~1.7e-2
    (gate is rel < 2e-2).
  - Dual-chain interleave: the DVE pays a ~100ns read-after-write stall on
    back-to-back dependent instructions, so each core's samples are split
    into two column-half chains (A = per-partition samples 0:32, B = 32:64)
    whose op streams interleave.  Consecutive instructions are independent
    (same-chain dependency distance 2), hiding the stall: measured ~16%
    faster than the single full-width chain.  Per-chain state is
    [S_x | C_x] (64 cols); pin/stage layouts are [A-block | B-block].
  - Output: interval results are written into a [128, G*128] staging tile
    (G=4 intervals per buffer); one DMA per full buffer into
    out[199,128,128].  This cuts SP-sequencer DMA-issue time ~4x vs
    per-interval DMAs.  Host unpacks, computes I and R, and transposes
    into [B,200,3].
"""

import numpy as np

try:
    import concourse.bass as bass
except ImportError:  # pragma: no cover - container default location
    import sys

    sys.path.insert(0, "/opt/trn_rl_repo")
    import concourse.bass as bass

import concourse.bacc as bacc
import concourse.mybir as mybir
from concourse.tile import TileContext
from concourse.bass_utils import run_bass_kernel_spmd

F32 = mybir.dt.float32
AL = mybir.AluOpType


def _register_ti_op():
    """Register a custom DVE op computing X = [t | I] from Y = [S | C] in ONE
    wide instruction: in0 = Y, in1 = column-block-swapped Y (= [C | S]).
    With r = Src1 - Src0:
      k <  64 (Src0=S, Src1=C): out = r*Src0 = (C-S)*S = S*I   (t half)
      k >= 64 (Src0=C, Src1=S): out = 0-r    = C-S = I         (I half)
    Bit-identical to the separate subtract+mult pair it replaces."""
    import numpy as _np
    from concourse import dve_ops as _dve_ops
    from concourse.dve_spec import Spec, Src0, Src1, C0, Zero, Idx, select, lower
    from concourse.dve_uop import DveOpSpec

    name = "SIR_TI_FUSED"
    for op in _dve_ops.OPS:
        if op.name == name:
            return op
    r = Src1 - Src0

    def _ref(in0, in1, s0):
        idx = _np.arange(in0.shape[-1], dtype=_np.float32)
        rr = in1 - in0
        return _np.where(idx < s0, rr * in0, -rr)

    spec = Spec(body=select(Idx < C0, r * Src0, Zero - r), reference=_ref)
    row = _dve_ops._CUSTOM_DVE_ROW_BASE + len(_dve_ops.OPS)
    assert row < 0x20
    shas = {
        ver: DveOpSpec(
            name=name, opcode=row, uops=lower(spec, ver=ver), rd1_en=True
        ).sha(ver)
        for ver in ("v3", "v4")
    }
    op = _dve_ops.DveOp(name, spec, subdim=False, uops_sha=shas)
    _dve_ops.OPS.append(op)
    _dve_ops.CUSTOM_DVE_SPECS[name] = spec
    _dve_ops._SUB_OPCODE_FOR_NAME[name] = row
    return op


_TI_OP = _register_ti_op()

N_CORES = 8
B = 65536
PER = B // N_CORES  # 8192 samples per core
P = 128
F = PER // P  # 64
NUM_T = 200
NI = NUM_T - 1  # 199 intervals
G = 8  # intervals per output staging buffer / DMA

# Bit-exact fp32 dt values of jnp.linspace(0, 100, 200, float32) diffs.
_DT_BITS = [
    0x3F00A4AA, 0x3F00A4AA, 0x3F00A4AA, 0x3F00A4AA, 0x3F00A4A8, 0x3F00A4AC, 0x3F00A4AC, 0x3F00A4A8, 0x3F00A4A8, 0x3F00A4A8,
    0x3F00A4B0, 0x3F00A4A8, 0x3F00A4A8, 0x3F00A4B0, 0x3F00A4A8, 0x3F00A4A8, 0x3F00A4B0, 0x3F00A4A0, 0x3F00A4B0, 0x3F00A4A0,
    0x3F00A4B0, 0x3F00A4B0, 0x3F00A4A0, 0x3F00A4B0, 0x3F00A4B0, 0x3F00A4A0, 0x3F00A4B0, 0x3F00A4B0, 0x3F00A4A0, 0x3F00A4B0,
    0x3F00A4A0, 0x3F00A4B0, 0x3F00A4A0, 0x3F00A4C0, 0x3F00A4A0, 0x3F00A4A0, 0x3F00A4C0, 0x3F00A4A0, 0x3F00A4A0, 0x3F00A4A0,
    0x3F00A4C0, 0x3F00A4A0, 0x3F00A4A0, 0x3F00A4C0, 0x3F00A4A0, 0x3F00A4A0, 0x3F00A4C0, 0x3F00A4A0, 0x3F00A4A0, 0x3F00A4C0,
    0x3F00A4A0, 0x3F00A4A0, 0x3F00A4C0, 0x3F00A4A0, 0x3F00A4A0, 0x3F00A4C0, 0x3F00A4A0, 0x3F00A4A0, 0x3F00A4A0, 0x3F00A4C0,
    0x3F00A4A0, 0x3F00A4A0, 0x3F00A4C0, 0x3F00A4A0, 0x3F00A4C0, 0x3F00A480, 0x3F00A4C0, 0x3F00A4C0, 0x3F00A480, 0x3F00A4C0,
    0x3F00A4C0, 0x3F00A480, 0x3F00A4C0, 0x3F00A4C0, 0x3F00A480, 0x3F00A4C0, 0x3F00A4C0, 0x3F00A480, 0x3F00A4C0, 0x3F00A480,
    0x3F00A4C0, 0x3F00A4C0, 0x3F00A480, 0x3F00A4C0, 0x3F00A4C0, 0x3F00A480, 0x3F00A4C0, 0x3F00A4C0, 0x3F00A480, 0x3F00A4C0,
    0x3F00A4C0, 0x3F00A480, 0x3F00A4C0, 0x3F00A4C0, 0x3F00A480, 0x3F00A4C0, 0x3F00A4C0, 0x3F00A480, 0x3F00A4C0, 0x3F00A4C0,
    0x3F00A480, 0x3F00A4C0, 0x3F00A4C0, 0x3F00A480, 0x3F00A4C0, 0x3F00A4C0, 0x3F00A480, 0x3F00A4C0, 0x3F00A4C0, 0x3F00A480,
    0x3F00A4C0, 0x3F00A4C0, 0x3F00A480, 0x3F00A4C0, 0x3F00A480, 0x3F00A4C0, 0x3F00A4C0, 0x3F00A480, 0x3F00A4C0, 0x3F00A4C0,
    0x3F00A480, 0x3F00A4C0, 0x3F00A4C0, 0x3F00A480, 0x3F00A4C0, 0x3F00A4C0, 0x3F00A480, 0x3F00A4C0, 0x3F00A480, 0x3F00A500,
    0x3F00A480, 0x3F00A480, 0x3F00A500, 0x3F00A480, 0x3F00A480, 0x3F00A500, 0x3F00A480, 0x3F00A480, 0x3F00A500, 0x3F00A480,
    0x3F00A480, 0x3F00A500, 0x3F00A480, 0x3F00A480, 0x3F00A500, 0x3F00A480, 0x3F00A480, 0x3F00A500, 0x3F00A480, 0x3F00A480,
    0x3F00A500, 0x3F00A480, 0x3F00A480, 0x3F00A500, 0x3F00A480, 0x3F00A480, 0x3F00A500, 0x3F00A480, 0x3F00A480, 0x3F00A480,
    0x3F00A500, 0x3F00A480, 0x3F00A480, 0x3F00A500, 0x3F00A480, 0x3F00A480, 0x3F00A500, 0x3F00A480, 0x3F00A480, 0x3F00A500,
    0x3F00A480, 0x3F00A480, 0x3F00A500, 0x3F00A480, 0x3F00A480, 0x3F00A500, 0x3F00A480, 0x3F00A480, 0x3F00A500, 0x3F00A480,
    0x3F00A480, 0x3F00A500, 0x3F00A480, 0x3F00A480, 0x3F00A500, 0x3F00A480, 0x3F00A480, 0x3F00A500, 0x3F00A480, 0x3F00A480,
    0x3F00A500, 0x3F00A480, 0x3F00A480, 0x3F00A500, 0x3F00A480, 0x3F00A480, 0x3F00A500, 0x3F00A480, 0x3F00A480,
]
DTS = np.array(_DT_BITS, dtype=np.uint32).view(np.float32)
assert DTS.shape == (NI,)

# Integration schedule: (method, substeps) per interval.  See module
# docstring; error vs fp32 reference rel ~2.4e-3 / absmax ~1.7e-2
# (sched_sim.py), gate is rel < 2e-2.
AB2S2 = 8
SCHEDULE = [("mid", 2)] + [("ab2", 2)] * AB2S2 + [("ab2", 1)] * (NI - 1 - AB2S2)
assert len(SCHEDULE) == NI

H = F // 2  # 32 cols per chain per state variable

# dtype for the X/K intermediate tiles (state Y stays f32).  fp16 rounding
# of the stage derivatives is numerically negligible here (AB2 truncation
# error dominates; sim: rel unchanged at 3 digits).  With all-fp16 packed
# operands the K multiply uses tensor_tensor, whose 2x_1p DVE perf mode
# processes 2 elements/cycle.
F16 = mybir.dt.float16
KDT = F16


def _rev(ap):
    return ap.rearrange("p (two f) -> p two f", two=2)[:, ::-1, :]


def _eval_K(nc, pool, cstA, cstB, YA, YB, tag):
    """Stage derivative K = [-b*S*I | -g*I] for both chains (4 DVE ops,
    interleaved A/B so consecutive instructions are independent)."""
    v = nc.vector
    X = pool.tile([P, 2 * F], KDT, tag="X")
    XA, XB = X[:, 0 : 2 * H], X[:, 2 * H : 4 * H]
    v._custom_dve(_TI_OP, out=XA, in0=YA, in1=_rev(YA), s0=float(H))  # [t | I]
    v._custom_dve(_TI_OP, out=XB, in0=YB, in1=_rev(YB), s0=float(H))
    K = pool.tile([P, 2 * F], KDT, tag=tag)
    if KDT is F32:
        v.scalar_tensor_tensor(K[:, 0 : 2 * H], cstA, 1.0, XA, AL.mult, AL.mult)
        v.scalar_tensor_tensor(K[:, 2 * H : 4 * H], cstB, 1.0, XB, AL.mult, AL.mult)
    else:
        v.tensor_tensor(K[:, 0 : 2 * H], cstA, XA, AL.mult)
        v.tensor_tensor(K[:, 2 * H : 4 * H], cstB, XB, AL.mult)
    return K


def _sub_mid(nc, pool, cstA, cstB, YA, YB, YoutA, YoutB, h):
    """Midpoint RK2, both chains: 12 DVE ops.  Returns f(Y) (AB2 history)."""
    v = nc.vector
    c2 = float(h / np.float32(2.0))
    K1 = _eval_K(nc, pool, cstA, cstB, YA, YB, "Kab")
    Y2 = pool.tile([P, 2 * F], F32, tag="Y2")
    Y2A, Y2B = Y2[:, 0 : 2 * H], Y2[:, 2 * H : 4 * H]
    v.scalar_tensor_tensor(Y2A, K1[:, 0 : 2 * H], c2, YA, AL.mult, AL.add)
    v.scalar_tensor_tensor(Y2B, K1[:, 2 * H : 4 * H], c2, YB, AL.mult, AL.add)
    K2 = _eval_K(nc, pool, cstA, cstB, Y2A, Y2B, "K2")
    v.scalar_tensor_tensor(YoutA, K2[:, 0 : 2 * H], float(h), YA, AL.mult, AL.add)
    v.scalar_tensor_tensor(YoutB, K2[:, 2 * H : 4 * H], float(h), YB, AL.mult, AL.add)
    return K1


def _sub_ab2(nc, pool, cstA, cstB, YA, YB, YoutA, YoutB, kprev, a, brat):
    """Variable-step Adams-Bashforth 2, both chains: 8 DVE ops.
    y+ = y + a*(k_n + brat*k_{n-1}),  a = h_n(1+r/2), brat = -(r/2)/(1+r/2),
    r = h_n/h_{n-1}.  Returns k_n (next step's history)."""
    v = nc.vector
    K = _eval_K(nc, pool, cstA, cstB, YA, YB, "Kab")
    Bt = pool.tile([P, 2 * F], F32, tag="B")
    v.scalar_tensor_tensor(
        Bt[:, 0 : 2 * H], kprev[:, 0 : 2 * H], brat, K[:, 0 : 2 * H],
        AL.mult, AL.add,
    )
    v.scalar_tensor_tensor(
        Bt[:, 2 * H : 4 * H], kprev[:, 2 * H : 4 * H], brat, K[:, 2 * H : 4 * H],
        AL.mult, AL.add,
    )
    v.scalar_tensor_tensor(YoutA, Bt[:, 0 : 2 * H], a, YA, AL.mult, AL.add)
    v.scalar_tensor_tensor(YoutB, Bt[:, 2 * H : 4 * H], a, YB, AL.mult, AL.add)
    return K


def _emit_body(nc, cpool, spool, wpool, pin, out_fn):
    """Emit one full integration.  out_fn(k0, width_ivals, stage_tile) is
    called to emit the output DMA for intervals [k0, k0+width_ivals)."""

    def body(_=None):
        pint = cpool.tile([P, 4 * F], F32, tag="pin")
        nc.sync.dma_start(out=pint[:], in_=pin[:])
        cstA = pint[:, 0 : 2 * H]  # [-bA | -gA]
        cstB = pint[:, 2 * H : 4 * H]  # [-bB | -gB]
        if KDT is not F32:
            c16 = cpool.tile([P, 4 * H], KDT, tag="c16")
            nc.vector.tensor_copy(c16[:], pint[:, 0 : 4 * H])
            cstA = c16[:, 0 : 2 * H]
            cstB = c16[:, 2 * H : 4 * H]
        YA = pint[:, 4 * H : 6 * H]  # [SA | CA]
        YB = pint[:, 6 * H : 8 * H]  # [SB | CB]
        kprev = None
        h_prev = None
        stage = None
        k0 = 0
        for k in range(NI):
            slot = k % G
            if slot == 0:
                stage = spool.tile([P, G * 2 * F], F32, tag="stage")
                k0 = k
            base = slot * 2 * F
            YsA = stage[:, base : base + 2 * H]
            YsB = stage[:, base + 2 * H : base + 4 * H]
            meth, nsub = SCHEDULE[k]
            h = np.float32(DTS[k]) / np.float32(nsub)
            for s in range(nsub):
                if s == nsub - 1:
                    YoA, YoB = YsA, YsB
                else:
                    Ymid = wpool.tile([P, 2 * F], F32, tag="Ymid")
                    YoA, YoB = Ymid[:, 0 : 2 * H], Ymid[:, 2 * H : 4 * H]
                if meth == "mid":
                    kprev = _sub_mid(nc, wpool, cstA, cstB, YA, YB, YoA, YoB, h)
                    h_prev = float(h)
                elif meth == "ab2":
                    hn = float(h)
                    r = hn / h_prev
                    a = float(np.float32(hn * (1 + r / 2)))
                    brat = float(np.float32(-(r / 2) / (1 + r / 2)))
                    kprev = _sub_ab2(
                        nc, wpool, cstA, cstB, YA, YB, YoA, YoB, kprev, a, brat
                    )
                    h_prev = hn
                else:
                    raise ValueError(meth)
                YA, YB = YoA, YoB
            if slot == G - 1 or k == NI - 1:
                out_fn(k0, slot + 1, stage)

    return body


def _declare_and_build(nc, reps, out_dram, live_out=None):
    pin = nc.declare_dram_parameter("pin", [P, 4 * F], F32, isOutput=False)

    with TileContext(nc) as tc:
        with (
            tc.tile_pool(name="const", bufs=1) as cpool,
            tc.tile_pool(name="stage", bufs=3) as spool,
            tc.tile_pool(name="work", bufs=3) as wpool,
        ):

            def out_fn(k0, nival, stage):
                # keep the partition axis outermost on the SBUF side; the
                # DRAM side takes the matching strided view
                src = stage[:, 0 : nival * 2 * F].rearrange(
                    "p (g f) -> p g f", g=nival
                )
                dst = out_dram[k0 : k0 + nival].rearrange("g p f -> p g f")
                nc.sync.dma_start(out=dst, in_=src)

            body = _emit_body(nc, cpool, spool, wpool, pin, out_fn)
            if reps == 1:
                body()
            else:
                # timing mode: repeat the whole kernel body inside one NEFF so
                # per-rep HW time can be separated from dispatch overhead
                with tc.For_i(0, reps, 1):
                    body()
            if live_out is not None:
                # graph liveness for the timing twin: route one scratch tile
                # to the tiny external output after the loop
                fin = cpool.tile([P, 2 * F], F32, tag="fin")
                nc.sync.dma_start(out=fin[:], in_=out_dram[NI - 1])
                nc.sync.dma_start(out=live_out[:], in_=fin[:])
    # run_bass_via_pjrt does not finalize; Bacc needs it (register alloc +
    # sync-wait splitting happen in its compile() pipeline).
    nc.finalize()
    return nc


def build_nc(reps=1):
    # Bacc (not raw Bass): its compile() pipeline runs generate_event_semaphores,
    # which splits multi-wait sync conditions that TRN2 instructions can't carry.
    nc = bacc.Bacc(None)
    out = nc.declare_dram_parameter("out", [NI, P, 2 * F], F32, isOutput=True)
    return _declare_and_build(nc, reps, out)


def build_timing(reps):
    """Timing twin: trajectory goes to internal DRAM scratch (no 104MB host
    zero-out transfer per call), tiny external output keeps the graph live."""
    nc = bacc.Bacc(None)
    outs = nc.declare_dram_parameter("outs", [P, 2 * F], F32, isOutput=True)
    scratch = nc.dram_tensor("traj", [NI, P, 2 * F], F32, kind="Internal")
    return _declare_and_build(nc, reps, scratch, live_out=outs)


_NC_CACHE = {}


def kernel(params: np.ndarray) -> np.ndarray:
    params = np.asarray(params, dtype=np.float32)
    assert params.shape == (B, 4)

    if "nc" not in _NC_CACHE:
        _NC_CACHE["nc"] = build_nc()
    nc = _NC_CACHE["nc"]

    in_maps = []
    for c in range(N_CORES):
        sl = params[c * PER : (c + 1) * PER]
        rb = (-sl[:, 0]).reshape(P, F)  # -beta
        rg = (-sl[:, 1]).reshape(P, F)  # -gamma
        rs = sl[:, 2].reshape(P, F)  # S0
        rc = (sl[:, 2] + sl[:, 3]).reshape(P, F)  # C0 = S0+I0
        # chain-blocked layout: [bA|gA|bB|gB | SA|CA|SB|CB], blocks of H=32
        Hh = F // 2
        pin = np.empty((P, 4 * F), dtype=np.float32)
        pin[:, 0:Hh] = rb[:, :Hh]
        pin[:, Hh : 2 * Hh] = rg[:, :Hh]
        pin[:, 2 * Hh : 3 * Hh] = rb[:, Hh:]
        pin[:, 3 * Hh : 4 * Hh] = rg[:, Hh:]
        pin[:, 4 * Hh : 5 * Hh] = rs[:, :Hh]
        pin[:, 5 * Hh : 6 * Hh] = rc[:, :Hh]
        pin[:, 6 * Hh : 7 * Hh] = rs[:, Hh:]
        pin[:, 7 * Hh : 8 * Hh] = rc[:, Hh:]
        in_maps.append({"pin": pin})

    res = run_bass_kernel_spmd(nc, in_maps, list(range(N_CORES)))

    out_full = np.empty((B, NUM_T, 3), dtype=np.float32)
    one = np.float32(1.0)
    S0 = params[:, 2]
    I0 = params[:, 3]
    out_full[:, 0, 0] = S0
    out_full[:, 0, 1] = I0
    out_full[:, 0, 2] = (one - S0) - I0
    for c in range(N_CORES):
        o = res.results[c]["out"]  # [NI, P, 2F] = [.., SA|CA|SB|CB]
        ob = o.reshape(NI, P, 4, F // 2)
        S = ob[:, :, (0, 2), :].reshape(NI, PER).T  # [PER, NI]
        C = ob[:, :, (1, 3), :].reshape(NI, PER).T
        blk = out_full[c * PER : (c + 1) * PER]
        blk[:, 1:, 0] = S
        blk[:, 1:, 1] = C - S
        blk[:, 1:, 2] = one - C
    return out_full


if __name__ == "__main__":
    rng = np.random.RandomState(0)
    p = rng.uniform(0, 1, (B, 4)).astype(np.float32)
    r = kernel(p)
    print(r.shape, r.dtype, r[0, :3], flush=True)
